# revision 31
# baseline (speedup 1.0000x reference)
"""MoD wrapper (router -> top-k -> gather -> GELU MLP -> weighted scatter-add)
on 8 Trainium2 NeuronCores.

Sharding: data-parallel over batch (4 sequences) x 2-way split of each
sequence's K=2048 selected tokens -> 8 cores, 1024 tokens each. Every core
holds the full FFN weights and computes
    y[t, :] = gate[t] * gelu_tanh(x[t, :] @ w1 + b1) @ w2
for its 1024 tokens. Routing (scores / top-k / sigmoid) runs through the
same jax ops the reference uses, so token selection matches the reference
bit-for-bit; gather and the final scatter-add into the residual stream are
host-side numpy (b2 is folded into the scatter).

Matmuls run in fp8e4 (IEEE e4m3) with DoubleRow perf mode (2 fp8 weights
per PE cell -> 2x MAC throughput). Operands are pre-scaled by powers of
two so their magnitudes sit in e4m3's normal range; the dequant scales are
folded into the GELU activation scale (phase 1) and the host-side gate
multiply (phase 2), so no extra device instructions are spent on scaling.
Phase 2 computes yT = w2.T @ hT with w2 stationary (each weight tile
DMA'd once) and hT streaming from SBUF, so phase 2 does half the input
DMA of the natural orientation and returns y transposed (in bf16; the
fp32->bf16 cast rides the existing psum-drain copy and halves the output
DMA).

Schedule notes (from perfetto traces): the kernel is tensor-bound at
~95% of the fp8-DoubleRow peak (2048 matmuls x ~216ns). The lead-in is
limited by DMA descriptor generation (~711ns per 256KB tile, serialized
per HWDGE ring), so the first psum group's operands are pre-issued
split across BOTH rings (w1 on sync, xT + b1 on scalar) with the k0
tiles further chunked per DoubleRow step. hT lives in 16 per-k-tile
SBUF tiles so phase-2 reads depend only on the GELUs that produced
that slab.
"""

import sys
import types

import numpy as np
import ml_dtypes

# bass_utils' trace path does `from antenv.axon_hooks import ...`; some
# images ship an antenv without that module (boot degrades silently but the
# import in bass_utils would crash). Register a no-op stand-in so trace=True
# degrades to "no profile" instead of raising.
try:
    import antenv.axon_hooks  # noqa: F401
except Exception:
    import antenv

    _hooks = types.ModuleType("antenv.axon_hooks")
    _hooks._hook = None
    _hooks.set_axon_ntff_profile_hook = \
        lambda h: setattr(_hooks, "_hook", h)
    _hooks.get_axon_ntff_profile_hook = \
        lambda: getattr(_hooks, "_hook", None)
    sys.modules["antenv.axon_hooks"] = _hooks
    antenv.axon_hooks = _hooks
    try:
        # Same registration trn_agent_boot.boot() would have done had the
        # module existed at interpreter start.
        from trn_agent_boot.trn_boot import _ntff_profile_via_ctypes

        _hook = _ntff_profile_via_ctypes("/opt/axon/libaxon_pjrt.so")
        if _hook is not None:
            _hooks.set_axon_ntff_profile_hook(_hook)
    except Exception:
        pass

import concourse.bacc as bacc
import concourse.bass as bass
import concourse.mybir as mybir
import concourse.tile as tile
from concourse.bass import ts
from concourse.bass_utils import run_bass_kernel_spmd
from concourse.kernels.tile_matmul import (
    ShapeInfo,
    TileKxM,
    TileKxN,
    batched_consumer,
    batched_producer_kxm,
    composable_matmul_tile_kernel,
    dma_from_dram_kxm,
    dma_from_dram_kxn,
    dma_to_dram_mxn,
)

B, S, D, DFF = 4, 4096, 2048, 8192
K = 2048          # selected tokens per sequence
N_CORES = 8
TPC = (B * K) // N_CORES  # tokens per core = 1024

BF16 = mybir.dt.bfloat16
FP8 = mybir.dt.float8e4
F32 = mybir.dt.float32
P = 128

E4M3 = ml_dtypes.float8_e4m3  # IEEE e4m3 (max 240) == TRN FP8_EXP4

# Per-phase operand dtypes. fp8e4 tiles auto-enable DoubleRow inside
# composable_matmul_tile_kernel (2x tensor-engine throughput).
PH1_FP8 = True   # x, w1 operands (h produced by GELU)
PH2_FP8 = True   # h, w2 operands

# Power-of-two pre-scales so fp8 operands use e4m3's normal range
# (w1/w2 entries are ~N(0, 1/sqrt(fan_in)) -- raw values would land in
# the subnormal range and lose mantissa bits).
S_X = 16.0 if PH1_FP8 else 1.0
S_W1 = 128.0 if PH1_FP8 else 1.0
S_W2 = 256.0 if PH2_FP8 else 1.0


def _build_nc():
    nc = bacc.Bacc("TRN2", target_bir_lowering=False, debug=False,
                   num_devices=N_CORES)

    in_dt1 = FP8 if PH1_FP8 else BF16
    in_dt2 = FP8 if PH2_FP8 else BF16

    xT_ap = nc.dram_tensor("xT", [D, TPC], in_dt1, kind="ExternalInput").ap()
    w1_ap = nc.dram_tensor("w1", [D, DFF], in_dt1, kind="ExternalInput").ap()
    w2_ap = nc.dram_tensor("w2", [DFF, D], in_dt2, kind="ExternalInput").ap()
    b1_ap = nc.dram_tensor("b1v", [P, DFF // P], F32, kind="ExternalInput").ap()
    y_ap = nc.dram_tensor("y", [D, TPC], BF16, kind="ExternalOutput").ap()

    with tile.TileContext(nc) as tc:
        with (
            tc.tile_pool(name="const", bufs=1) as const_pool,
            tc.tile_pool(name="hT", bufs=1) as hT_pool,
            tc.tile_pool(name="kxm1", bufs=9) as kxm1_pool,
            tc.tile_pool(name="kxn1", bufs=9) as kxn1_pool,
            tc.tile_pool(name="kxm2", bufs=18) as kxm2_pool,
        ):
            # Intermediate hT[f, t] = gelu(w1.T @ x.T + b1), kept in SBUF
            # as the kxn operand of the second matmul. Split into one tile
            # per phase-1 m-tile (= phase-2 k-tile) so phase 2's k_tile i
            # depends only on the GELUs that wrote slab i — with a single
            # big tile, the tile-granular dependency made phase 2's first
            # matmul wait for the LAST phase-1 GELU (a ~1.2us stall).
            hT_tiles = [
                hT_pool.tile([P, 4, TPC], in_dt2, name=f"hT{i}")
                for i in range(DFF // 512)
            ]

            # ---- phase 1: hT = gelu((w1.T @ xT) / (S_X*S_W1) + b1) ----
            _w1_dma, kxm1_shape = dma_from_dram_kxm(kxm1_pool, w1_ap)
            _kxn1_dma, kxn1_shape = dma_from_dram_kxn(kxn1_pool, xT_ap)

            # Pre-issue the first psum group's full operand set (k tiles
            # 0..3 of w1 m-tile 0 and of xT n-tile 0), split across BOTH
            # HWDGE rings (w1 on sync, xT + b1 on scalar) so descriptor
            # generation — the lead-in bottleneck at ~711ns per 256KB tile,
            # serialized per ring — runs 2x parallel. The k0 tiles are
            # further split into DoubleRow-pair chunks so the very first
            # matmul's operands land in ~1us. Tags match _dma_from_dram's
            # so the pool ring accounting stays uniform.
            w1_3d = w1_ap.rearrange("(po pi) f -> pi po f", pi=P)
            xT_3d = xT_ap.rearrange("(po pi) f -> pi po f", pi=P)
            tag = f"tile_128_4_512_{in_dt1}"
            _w1_seeds = {}
            _xt_seeds = {}
            b1_sb = const_pool.tile([P, DFF // P], F32)
            for k in range(4):
                wt = kxm1_pool.tile([P, 4, 512], in_dt1, tag=tag,
                                    name=f"w1_pre{k}")
                xt = kxn1_pool.tile([P, 4, 512], in_dt1, tag=tag,
                                    name=f"xt_pre{k}")
                if k == 0:
                    nc.sync.dma_start(wt[:, 0:2, :], w1_3d[:, 0:2, 0:512])
                    nc.scalar.dma_start(xt[:, 0:2, :], xT_3d[:, 0:2, 0:512])
                    nc.sync.dma_start(wt[:, 2:4, :], w1_3d[:, 2:4, 0:512])
                    nc.scalar.dma_start(xt[:, 2:4, :], xT_3d[:, 2:4, 0:512])
                    # b1 early on the scalar ring (needed by the first GELU
                    # at ~13us; the gpsimd SWDGE ring was delivering it at
                    # ~9.5us, uncomfortably late).
                    nc.scalar.dma_start(b1_sb[:], b1_ap[:])
                else:
                    nc.sync.dma_start(
                        wt[:], w1_3d[:, ts(k, 4), 0:512])
                    nc.scalar.dma_start(
                        xt[:], xT_3d[:, ts(k, 4), 0:512])
                _w1_seeds[(0, k)] = wt
                _xt_seeds[(k, 0)] = xt

            # Also seed xT's n1 tiles on the scalar ring (idle after the
            # n0 seeds): emitted by the main flow they queue on the sync
            # ring BEHIND the w1 seeds and land ~1.2us after block 1
            # needs them; here they land ~10.5us, and w1's m1 tiles move
            # ~3us earlier on the sync ring.
            for k in range(4):
                xt1 = kxn1_pool.tile([P, 4, 512], in_dt1, tag=tag,
                                     name=f"xt1_pre{k}")
                nc.scalar.dma_start(xt1[:], xT_3d[:, ts(k, 4), 512:1024])
                _xt_seeds[(k, 1)] = xt1

            # xT is only 1MB: memoize the 8 (k, n) tiles at build time so
            # each is DMA'd once and stays resident, instead of being
            # re-fetched every m-block (8.5x redundant traffic + ~60 extra
            # sync-queue DMA triggers). Pool bufs=9 > 8 live tiles, so the
            # pool never recycles them.
            _xt_tiles = dict(_xt_seeds)

            def kxn1_producer(nc_, md):
                key = (md.k_tile_idx, md.n_tile_idx)
                if key not in _xt_tiles:
                    _xt_tiles[key] = _kxn1_dma(nc_, md)
                return _xt_tiles[key]

            def kxm1_producer(nc_, md):
                t = _w1_seeds.pop((md.m_tile_idx, md.k_tile_idx), None)
                return t if t is not None else _w1_dma(nc_, md)

            def hT_slice_producer(nc_, md):
                return hT_tiles[md.m_tile_idx][:, :, md.n_slice]

            inv_s1 = 1.0 / (S_X * S_W1)

            def gelu_reducer(nc_, psum, sbuf, md):
                f_outer = md.m_tile_idx * md.m_subtiles + md.m_subtile_idx
                nc_.scalar.activation(
                    sbuf,
                    psum,
                    mybir.ActivationFunctionType.Gelu_apprx_tanh,
                    bias=b1_sb[:, f_outer:f_outer + 1],
                    scale=inv_s1,
                )

            composable_matmul_tile_kernel(
                tc,
                kxm_shape=kxm1_shape,
                kxn_shape=kxn1_shape,
                output_type=None,
                kxm_producer=kxm1_producer,
                kxn_producer=kxn1_producer,
                mxn_consumer=lambda nc_, t, md: None,
                mxn_subtile_reducer=gelu_reducer,
                mxn_subtile_producer=hT_slice_producer,
                cache_tiles=True,
            )

            # ---- phase 2: yT = w2.T @ hT  (y transposed: [D, tokens]) ----
            # w2 is the stationary operand (M over D, each tile DMA'd once
            # and cached across the token n-tiles); hT streams from SBUF
            # with zero DMA. The per-token gate (and the 1/S_W2 dequant)
            # is applied host-side during the scatter-add.
            # M (= D) is split into batches (1920, 128) so the FINAL output
            # block is a single 128-row psum tile: its drain after the last
            # matmul is one copy + one DMA instead of several, which
            # shortens the kernel tail.
            M_SPLIT = 1920
            kxm2a, s2a = dma_from_dram_kxm(kxm2_pool, w2_ap[:, :M_SPLIT])
            kxm2b, s2b = dma_from_dram_kxm(kxm2_pool, w2_ap[:, M_SPLIT:])
            kxm2_producer, kxm2_shape = batched_producer_kxm(
                [kxm2a, kxm2b], [s2a, s2b], batch_dim="m")

            def hT_kxn_producer(nc_, md):
                return hT_tiles[md.k_tile_idx][:, :,
                                               ts(md.n_tile_idx, md.n_tile)]

            kxn2_shape = ShapeInfo(pdims=((P, DFF // P),), fdims=(TPC,))

            composable_matmul_tile_kernel(
                tc,
                kxm_shape=kxm2_shape,
                kxn_shape=kxn2_shape,
                output_type=BF16,
                kxm_producer=kxm2_producer,
                kxn_producer=hT_kxn_producer,
                mxn_consumer=batched_consumer(
                    [dma_to_dram_mxn(y_ap[:M_SPLIT]),
                     dma_to_dram_mxn(y_ap[M_SPLIT:])], batch_dim="m"),
                cache_tiles=True,
                psum_n_bufs=2,
            )

    nc.compile()
    return nc


_NC = None


def _routing(hidden_states, router_weight, router_bias):
    """Same ops/backend as the reference => bit-identical selection."""
    import jax
    import jax.numpy as jnp
    scores = jnp.einsum('bsd,d->bs', hidden_states, router_weight) \
        + router_bias[0]
    top_scores, indices = jax.lax.top_k(scores, K)
    weights = jax.nn.sigmoid(top_scores)
    return np.asarray(indices), np.asarray(weights)


def _q8(a, scale):
    """Scale, clip to e4m3's finite range, quantize."""
    return np.clip(a * scale, -240.0, 240.0).astype(E4M3)


def _run(hidden_states, router_weight, router_bias, w1, b1, w2, b2,
         trace=False):
    global _NC
    hidden_states = np.asarray(hidden_states, dtype=np.float32)
    router_weight = np.asarray(router_weight, dtype=np.float32)
    router_bias = np.asarray(router_bias, dtype=np.float32)
    w1 = np.asarray(w1, dtype=np.float32)
    b1 = np.asarray(b1, dtype=np.float32)
    w2 = np.asarray(w2, dtype=np.float32)
    b2 = np.asarray(b2, dtype=np.float32)

    indices, weights = _routing(hidden_states, router_weight, router_bias)

    if _NC is None:
        _NC = _build_nc()

    if PH1_FP8:
        w1_dev = _q8(w1, S_W1)
    else:
        w1_dev = w1.astype(ml_dtypes.bfloat16)
    if PH2_FP8:
        w2_dev = _q8(w2, S_W2)
    else:
        w2_dev = w2.astype(ml_dtypes.bfloat16)
    b1v = np.ascontiguousarray(b1.reshape(DFF // P, P).T)

    in_maps = []
    core_idx = []  # (b, idx_slice, gate) per core
    for c in range(N_CORES):
        b, h = divmod(c, 2)
        idx_c = indices[b, h * TPC:(h + 1) * TPC]
        xT = hidden_states[b, idx_c].T
        if PH1_FP8:
            xT = _q8(xT, S_X)
        else:
            xT = xT.astype(ml_dtypes.bfloat16)
        in_maps.append({
            "xT": np.ascontiguousarray(xT),
            "w1": w1_dev,
            "w2": w2_dev,
            "b1v": b1v,
        })
        core_idx.append((b, idx_c, weights[b, h * TPC:(h + 1) * TPC]))

    res = run_bass_kernel_spmd(_NC, in_maps, core_ids=list(range(N_CORES)),
                               trace=trace)

    out = hidden_states.copy().reshape(B * S, D)
    b2_nonzero = bool(np.any(b2))
    for c in range(N_CORES):
        b, idx_c, gate_c = core_idx[c]
        # y comes back transposed [D, TPC] in bf16; gate + 1/S_W2 dequant
        # fold in during the scatter.
        y = res.results[c]["y"].astype(np.float32).T * (gate_c / S_W2)[:, None]
        if b2_nonzero:
            y = y + gate_c[:, None] * b2[None, :]
        out[b * S + idx_c] += y
    return out.reshape(B, S, D), res


def kernel(**inputs):
    return _run(**inputs)[0]



# revision 33
# speedup vs baseline: 1.0148x; 1.0148x over previous
"""MoD wrapper (router -> top-k -> gather -> GELU MLP -> weighted scatter-add)
on 8 Trainium2 NeuronCores.

Sharding: data-parallel over batch (4 sequences) x 2-way split of each
sequence's K=2048 selected tokens -> 8 cores, 1024 tokens each. Every core
holds the full FFN weights and computes
    y[t, :] = gate[t] * gelu_tanh(x[t, :] @ w1 + b1) @ w2
for its 1024 tokens. Routing (scores / top-k / sigmoid) runs through the
same jax ops the reference uses, so token selection matches the reference
bit-for-bit; gather and the final scatter-add into the residual stream are
host-side numpy (b2 is folded into the scatter).

Matmuls run in fp8e4 (IEEE e4m3) with DoubleRow perf mode (2 fp8 weights
per PE cell -> 2x MAC throughput). Operands are pre-scaled by powers of
two so their magnitudes sit in e4m3's normal range; the dequant scales are
folded into the GELU activation scale (phase 1) and the host-side gate
multiply (phase 2), so no extra device instructions are spent on scaling.
Phase 2 computes yT = w2.T @ hT with w2 stationary (each weight tile
DMA'd once) and hT streaming from SBUF, so phase 2 does half the input
DMA of the natural orientation and returns y transposed (in bf16; the
fp32->bf16 cast rides the existing psum-drain copy and halves the output
DMA).

Schedule notes (from perfetto traces): the kernel is tensor-bound at
~95% of the fp8-DoubleRow peak (2048 matmuls x ~216ns). The lead-in is
limited by DMA descriptor generation (~711ns per 256KB tile, serialized
per HWDGE ring), so the first psum group's operands are pre-issued
split across BOTH rings (w1 on sync, xT + b1 on scalar) with the k0
tiles further chunked per DoubleRow step. hT lives in 16 per-k-tile
SBUF tiles so phase-2 reads depend only on the GELUs that produced
that slab.
"""

import sys
import types

import numpy as np
import ml_dtypes

# bass_utils' trace path does `from antenv.axon_hooks import ...`; some
# images ship an antenv without that module (boot degrades silently but the
# import in bass_utils would crash). Register a no-op stand-in so trace=True
# degrades to "no profile" instead of raising.
try:
    import antenv.axon_hooks  # noqa: F401
except Exception:
    import antenv

    _hooks = types.ModuleType("antenv.axon_hooks")
    _hooks._hook = None
    _hooks.set_axon_ntff_profile_hook = \
        lambda h: setattr(_hooks, "_hook", h)
    _hooks.get_axon_ntff_profile_hook = \
        lambda: getattr(_hooks, "_hook", None)
    sys.modules["antenv.axon_hooks"] = _hooks
    antenv.axon_hooks = _hooks
    try:
        # Same registration trn_agent_boot.boot() would have done had the
        # module existed at interpreter start.
        from trn_agent_boot.trn_boot import _ntff_profile_via_ctypes

        _hook = _ntff_profile_via_ctypes("/opt/axon/libaxon_pjrt.so")
        if _hook is not None:
            _hooks.set_axon_ntff_profile_hook(_hook)
    except Exception:
        pass

import concourse.bacc as bacc
import concourse.bass as bass
import concourse.mybir as mybir
import concourse.tile as tile
from concourse.bass import ts
from concourse.bass_utils import run_bass_kernel_spmd
from concourse.kernels.tile_matmul import (
    ShapeInfo,
    TileKxM,
    TileKxN,
    batched_consumer,
    batched_producer_kxm,
    composable_matmul_tile_kernel,
    dma_from_dram_kxm,
    dma_from_dram_kxn,
    dma_to_dram_mxn,
)

B, S, D, DFF = 4, 4096, 2048, 8192
K = 2048          # selected tokens per sequence
N_CORES = 8
TPC = (B * K) // N_CORES  # tokens per core = 1024

BF16 = mybir.dt.bfloat16
FP8 = mybir.dt.float8e4
F32 = mybir.dt.float32
P = 128

E4M3 = ml_dtypes.float8_e4m3  # IEEE e4m3 (max 240) == TRN FP8_EXP4

# Per-phase operand dtypes. fp8e4 tiles auto-enable DoubleRow inside
# composable_matmul_tile_kernel (2x tensor-engine throughput).
PH1_FP8 = True   # x, w1 operands (h produced by GELU)
PH2_FP8 = True   # h, w2 operands

# Power-of-two pre-scales so fp8 operands use e4m3's normal range
# (w1/w2 entries are ~N(0, 1/sqrt(fan_in)) -- raw values would land in
# the subnormal range and lose mantissa bits).
S_X = 16.0 if PH1_FP8 else 1.0
S_W1 = 128.0 if PH1_FP8 else 1.0
S_W2 = 256.0 if PH2_FP8 else 1.0


def _build_nc():
    nc = bacc.Bacc("TRN2", target_bir_lowering=False, debug=False,
                   num_devices=N_CORES)

    in_dt1 = FP8 if PH1_FP8 else BF16
    in_dt2 = FP8 if PH2_FP8 else BF16

    xT_ap = nc.dram_tensor("xT", [D, TPC], in_dt1, kind="ExternalInput").ap()
    w1_ap = nc.dram_tensor("w1", [D, DFF], in_dt1, kind="ExternalInput").ap()
    w2_ap = nc.dram_tensor("w2", [DFF, D], in_dt2, kind="ExternalInput").ap()
    b1_ap = nc.dram_tensor("b1v", [P, DFF // P], F32, kind="ExternalInput").ap()
    y_ap = nc.dram_tensor("y", [D, TPC], BF16, kind="ExternalOutput").ap()

    with tile.TileContext(nc) as tc:
        with (
            tc.tile_pool(name="const", bufs=1) as const_pool,
            tc.tile_pool(name="hT", bufs=1) as hT_pool,
            tc.tile_pool(name="kxm1", bufs=9) as kxm1_pool,
            tc.tile_pool(name="kxn1", bufs=9) as kxn1_pool,
            tc.tile_pool(name="kxm2", bufs=18) as kxm2_pool,
        ):
            # Intermediate hT[f, t] = gelu(w1.T @ x.T + b1), kept in SBUF
            # as the kxn operand of the second matmul. Split into one tile
            # per phase-1 m-tile (= phase-2 k-tile) so phase 2's k_tile i
            # depends only on the GELUs that wrote slab i — with a single
            # big tile, the tile-granular dependency made phase 2's first
            # matmul wait for the LAST phase-1 GELU (a ~1.2us stall).
            hT_tiles = [
                hT_pool.tile([P, 4, TPC], in_dt2, name=f"hT{i}")
                for i in range(DFF // 512)
            ]

            # ---- phase 1: hT = gelu((w1.T @ xT) / (S_X*S_W1) + b1) ----
            _w1_dma, kxm1_shape = dma_from_dram_kxm(kxm1_pool, w1_ap)
            _kxn1_dma, kxn1_shape = dma_from_dram_kxn(kxn1_pool, xT_ap)

            # Pre-issue the first psum group's full operand set (k tiles
            # 0..3 of w1 m-tile 0 and of xT n-tile 0), split across BOTH
            # HWDGE rings (w1 on sync, xT + b1 on scalar) so descriptor
            # generation — the lead-in bottleneck at ~711ns per 256KB tile,
            # serialized per ring — runs 2x parallel. The k0 tiles are
            # further split into DoubleRow-pair chunks so the very first
            # matmul's operands land in ~1us. Tags match _dma_from_dram's
            # so the pool ring accounting stays uniform.
            w1_3d = w1_ap.rearrange("(po pi) f -> pi po f", pi=P)
            xT_3d = xT_ap.rearrange("(po pi) f -> pi po f", pi=P)
            tag = f"tile_128_4_512_{in_dt1}"
            _w1_seeds = {}
            _xt_seeds = {}
            b1_sb = const_pool.tile([P, DFF // P], F32)
            for k in range(4):
                wt = kxm1_pool.tile([P, 4, 512], in_dt1, tag=tag,
                                    name=f"w1_pre{k}")
                xt = kxn1_pool.tile([P, 4, 512], in_dt1, tag=tag,
                                    name=f"xt_pre{k}")
                if k == 0:
                    nc.sync.dma_start(wt[:, 0:2, :], w1_3d[:, 0:2, 0:512])
                    nc.scalar.dma_start(xt[:, 0:2, :], xT_3d[:, 0:2, 0:512])
                    nc.sync.dma_start(wt[:, 2:4, :], w1_3d[:, 2:4, 0:512])
                    nc.scalar.dma_start(xt[:, 2:4, :], xT_3d[:, 2:4, 0:512])
                    # b1 early on the scalar ring (needed by the first GELU
                    # at ~13us; the gpsimd SWDGE ring was delivering it at
                    # ~9.5us, uncomfortably late).
                    nc.scalar.dma_start(b1_sb[:], b1_ap[:])
                else:
                    nc.sync.dma_start(
                        wt[:], w1_3d[:, ts(k, 4), 0:512])
                    nc.scalar.dma_start(
                        xt[:], xT_3d[:, ts(k, 4), 0:512])
                _w1_seeds[(0, k)] = wt
                _xt_seeds[(k, 0)] = xt

            # xT is only 1MB: memoize the 8 (k, n) tiles at build time so
            # each is DMA'd once and stays resident, instead of being
            # re-fetched every m-block (8.5x redundant traffic + ~60 extra
            # sync-queue DMA triggers). Pool bufs=9 > 8 live tiles, so the
            # pool never recycles them.
            _xt_tiles = dict(_xt_seeds)

            def kxn1_producer(nc_, md):
                key = (md.k_tile_idx, md.n_tile_idx)
                if key not in _xt_tiles:
                    _xt_tiles[key] = _kxn1_dma(nc_, md)
                return _xt_tiles[key]

            def kxm1_producer(nc_, md):
                t = _w1_seeds.pop((md.m_tile_idx, md.k_tile_idx), None)
                return t if t is not None else _w1_dma(nc_, md)

            def hT_slice_producer(nc_, md):
                return hT_tiles[md.m_tile_idx][:, :, md.n_slice]

            inv_s1 = 1.0 / (S_X * S_W1)

            def gelu_reducer(nc_, psum, sbuf, md):
                f_outer = md.m_tile_idx * md.m_subtiles + md.m_subtile_idx
                nc_.scalar.activation(
                    sbuf,
                    psum,
                    mybir.ActivationFunctionType.Gelu_apprx_tanh,
                    bias=b1_sb[:, f_outer:f_outer + 1],
                    scale=inv_s1,
                )

            composable_matmul_tile_kernel(
                tc,
                kxm_shape=kxm1_shape,
                kxn_shape=kxn1_shape,
                output_type=None,
                kxm_producer=kxm1_producer,
                kxn_producer=kxn1_producer,
                mxn_consumer=lambda nc_, t, md: None,
                mxn_subtile_reducer=gelu_reducer,
                mxn_subtile_producer=hT_slice_producer,
                cache_tiles=True,
            )

            # ---- phase 2: yT = w2.T @ hT  (y transposed: [D, tokens]) ----
            # w2 is the stationary operand (M over D, each tile DMA'd once
            # and cached across the token n-tiles); hT streams from SBUF
            # with zero DMA. The per-token gate (and the 1/S_W2 dequant)
            # is applied host-side during the scatter-add.
            # M (= D) is split into batches (1920, 128) so the FINAL output
            # block is a single 128-row psum tile: its drain after the last
            # matmul is one copy + one DMA instead of several, which
            # shortens the kernel tail.
            M_SPLIT = 1920
            kxm2a, s2a = dma_from_dram_kxm(kxm2_pool, w2_ap[:, :M_SPLIT])
            kxm2b, s2b = dma_from_dram_kxm(kxm2_pool, w2_ap[:, M_SPLIT:])
            kxm2_producer, kxm2_shape = batched_producer_kxm(
                [kxm2a, kxm2b], [s2a, s2b], batch_dim="m")

            def hT_kxn_producer(nc_, md):
                return hT_tiles[md.k_tile_idx][:, :,
                                               ts(md.n_tile_idx, md.n_tile)]

            kxn2_shape = ShapeInfo(pdims=((P, DFF // P),), fdims=(TPC,))

            composable_matmul_tile_kernel(
                tc,
                kxm_shape=kxm2_shape,
                kxn_shape=kxn2_shape,
                output_type=BF16,
                kxm_producer=kxm2_producer,
                kxn_producer=hT_kxn_producer,
                mxn_consumer=batched_consumer(
                    [dma_to_dram_mxn(y_ap[:M_SPLIT]),
                     dma_to_dram_mxn(y_ap[M_SPLIT:])], batch_dim="m"),
                cache_tiles=True,
                psum_n_bufs=2,
            )

    nc.compile()
    return nc


_NC = None


def _routing(hidden_states, router_weight, router_bias):
    """Same ops/backend as the reference => bit-identical selection."""
    import jax
    import jax.numpy as jnp
    scores = jnp.einsum('bsd,d->bs', hidden_states, router_weight) \
        + router_bias[0]
    top_scores, indices = jax.lax.top_k(scores, K)
    weights = jax.nn.sigmoid(top_scores)
    return np.asarray(indices), np.asarray(weights)


def _q8(a, scale):
    """Scale, clip to e4m3's finite range, quantize."""
    return np.clip(a * scale, -240.0, 240.0).astype(E4M3)


def _run(hidden_states, router_weight, router_bias, w1, b1, w2, b2,
         trace=False):
    global _NC
    hidden_states = np.asarray(hidden_states, dtype=np.float32)
    router_weight = np.asarray(router_weight, dtype=np.float32)
    router_bias = np.asarray(router_bias, dtype=np.float32)
    w1 = np.asarray(w1, dtype=np.float32)
    b1 = np.asarray(b1, dtype=np.float32)
    w2 = np.asarray(w2, dtype=np.float32)
    b2 = np.asarray(b2, dtype=np.float32)

    indices, weights = _routing(hidden_states, router_weight, router_bias)

    if _NC is None:
        _NC = _build_nc()

    if PH1_FP8:
        w1_dev = _q8(w1, S_W1)
    else:
        w1_dev = w1.astype(ml_dtypes.bfloat16)
    if PH2_FP8:
        w2_dev = _q8(w2, S_W2)
    else:
        w2_dev = w2.astype(ml_dtypes.bfloat16)
    b1v = np.ascontiguousarray(b1.reshape(DFF // P, P).T)

    in_maps = []
    core_idx = []  # (b, idx_slice, gate) per core
    for c in range(N_CORES):
        b, h = divmod(c, 2)
        idx_c = indices[b, h * TPC:(h + 1) * TPC]
        xT = hidden_states[b, idx_c].T
        if PH1_FP8:
            xT = _q8(xT, S_X)
        else:
            xT = xT.astype(ml_dtypes.bfloat16)
        in_maps.append({
            "xT": np.ascontiguousarray(xT),
            "w1": w1_dev,
            "w2": w2_dev,
            "b1v": b1v,
        })
        core_idx.append((b, idx_c, weights[b, h * TPC:(h + 1) * TPC]))

    # One retry: a transient device wedge (NRT_EXEC_UNIT_UNRECOVERABLE was
    # observed once, from a previous tenant's run) would otherwise fail the
    # single kernel() invocation outright. Costs nothing on the success path.
    try:
        res = run_bass_kernel_spmd(_NC, in_maps,
                                   core_ids=list(range(N_CORES)), trace=trace)
    except Exception:
        import time as _time
        _time.sleep(5.0)
        res = run_bass_kernel_spmd(_NC, in_maps,
                                   core_ids=list(range(N_CORES)), trace=trace)

    out = hidden_states.copy().reshape(B * S, D)
    b2_nonzero = bool(np.any(b2))
    for c in range(N_CORES):
        b, idx_c, gate_c = core_idx[c]
        # y comes back transposed [D, TPC] in bf16; gate + 1/S_W2 dequant
        # fold in during the scatter.
        y = res.results[c]["y"].astype(np.float32).T * (gate_c / S_W2)[:, None]
        if b2_nonzero:
            y = y + gate_c[:, None] * b2[None, :]
        out[b * S + idx_c] += y
    return out.reshape(B, S, D), res


def kernel(**inputs):
    return _run(**inputs)[0]

